# revision 1
# baseline (speedup 1.0000x reference)
"""Trainium2 Bass kernel for nn_DiffHist (differentiable 256-bin histogram).

Contract: kernel(img) takes the FULL input img [128, 512, 512] f32 with
values in [0, 1], returns the FULL output h[256] f32 — identical math to
the reference:
    s = 255*img.ravel(); idx = floor(s); d = s - idx
    h[idx] += 1-d; h[idx+1] += d; return h[:256]

Strategy (data-parallel over 8 NeuronCores; each core gets 1/8 of the
flattened image as a [128, 32768] f32 block):

  Per core, the histogram is computed as a PSUM-accumulated bilinear
  form on the tensor engine.  With u = s/16 in [0, 16), coarse block
  a = floor(u) (16 blocks of 16 bins) and fine offset lo = 16*frac(u):

      h[16a + b] = sum_i [a_i == a] * tent(lo_i - b),  b = 0..16
      tent(d) = relu(1 - |d|) = relu(d+1) - 2 relu(d) + relu(d-1)

  Each chunk of 128 elements (one SBUF column) contributes one
  rank-128 update:  lhsT = U[k, a] = [a_k == a] (one-hot, 16 cols),
  rhs = V[k, p] = relu(lo_k - (p-1)) (ramp columns c = -1..17).  G=8
  chunks are packed per matmul (block-diagonal), so each matmul is
  lhsT [128, 128] x rhs [128, 152] accumulated into one PSUM tile; the tent
  second difference and the block-diagonal extraction happen on the
  host at gather time, as does the 8-way sum (the all-reduce of the
  per-core 272-float partial histograms).

  floor/frac are built with the fp32 magic-number trick
  (R = (u - 0.5) + 1.5*2^23) since the DVE has no floor/mod ALU op.

Numerics: U is exact {0,1}; lo is fp16 (|err| <= 2^-7 bin units) and V
ramps are fp16; PSUM accumulates in fp32.  Measured end-to-end relative
L2 error vs the fp64 reference is ~2e-5.
"""
import sys

sys.path.insert(0, '/opt/trn_rl_repo')

import numpy as np

# ----------------------------------------------------------------- tile patch
# The pinned walrus build accepts only one sync-wait command on several
# instruction classes; current concourse Tile attaches several to the
# kernel-tail drain and occasionally to DMA ops.  Split the excess waits
# onto dedicated single-wait instructions.
import bass_rust
import concourse.tile as tile
import concourse.mybir as mybir
from bass_rust import ScopedClock

_MAX_WAITS = 1


def _drain_and_barrier_split(self, tick_clock, wait_clock):
    nc = self.nc
    drain_inst = nc.sync.drain()
    wait_clock.add_sem_waits(
        drain_inst.ins, ScopedClock({None: tick_clock.global_clock})
    )
    si = drain_inst.ins.sync_info
    waits = list(si.on_wait) if si is not None and si.on_wait else []
    if len(waits) > _MAX_WAITS:
        drain_inst.ins.sync_info = bass_rust.SyncInfo(
            on_wait=waits[:_MAX_WAITS], on_update=list(si.on_update)
        )
        for w in waits[_MAX_WAITS:]:
            d2 = nc.sync.drain()
            d2.ins.sync_info = bass_rust.SyncInfo(on_wait=[w], on_update=[])
    nc.all_engine_barrier()
    assert self.sems is not None
    popped = nc._tile_sem_poison_stack.pop()
    assert popped is self._sem_poison
    nc.clear_and_free_semaphores(list(self.sems.allocated().values()))
    nc.all_engine_barrier()


def _split_excess_waits(nc, max_waits=_MAX_WAITS):
    for bb in nc.main_func.blocks:
        insts = list(bb.instructions)
        out = []
        changed = False
        for ins in insts:
            si = ins.sync_info
            if si is not None and si.on_wait and len(si.on_wait) > max_waits:
                waits = list(si.on_wait)
                extra, keep = waits[:-max_waits], waits[-max_waits:]
                for w in extra:
                    nop = mybir.InstNoOp(
                        name=f"waitnop-{nc.next_id()}",
                        engine=ins.engine,
                        bass_nofuse=True,
                        sync_info=mybir.SyncInfo(on_wait=[w], on_update=[]),
                    )
                    nc.register_instruction(nop, overwrite=True)
                    out.append(nop)
                ins.sync_info = bass_rust.SyncInfo(
                    on_wait=keep, on_update=list(si.on_update)
                )
                changed = True
            out.append(ins)
        if changed:
            bb.instructions = out


tile.TileContext._drain_and_barrier = _drain_and_barrier_split

# ----------------------------------------------------------------- kernel
import concourse.bass as bass

F32 = mybir.dt.float32
F16 = mybir.dt.float16
ALU = mybir.AluOpType
ACTF = mybir.ActivationFunctionType

NCORES = 8
NCOLS = 32768          # elements per partition per core
NA = 16                # coarse blocks
NB = 17                # relu ramp columns c = -1..15 (tent = 2nd diff)
G = 8                  # chunks per matmul
NOUT = NB * G          # 136
FD = 1024              # columns per tile
MAGIC = 12582912.0     # 1.5 * 2^23
N_V_ACT = 7            # V ramps on the scalar engine (ACT Relu)


def _build_nc():
    nc = bass.Bass()
    x = nc.declare_dram_parameter("x", [128, NCOLS], F32, isOutput=False)
    out = nc.declare_dram_parameter("hist", [128, NOUT], F32, isOutput=True)
    ntiles = NCOLS // FD

    # const APs for ACT Relu biases (mirrors Bass.__init__ register_const_ap)
    for cc in range(-1, 16):
        v = float(-cc)
        if (F32, v) not in nc.const_aps.aps:
            tcon = nc.alloc_sbuf_tensor(f"const-float32-{v}", [128, 1], F32)
            nc.gpsimd.memset(tcon.ap(), v)
            nc.const_aps.aps[(F32, v)] = tcon.ap()
    nc.all_engine_barrier()

    with tile.TileContext(nc) as tc:
        with (
            tc.tile_pool(name="sb", bufs=2) as sb,
            tc.tile_pool(name="sbo", bufs=1) as sbo,
            tc.tile_pool(name="psum", bufs=1, space="PSUM") as psum,
        ):
            acc = psum.tile([128, NOUT], F32)
            for t in range(ntiles):
                xt = sb.tile([128, FD], F32, tag="x")
                nc.sync.dma_start(xt[:], x[:, t * FD:(t + 1) * FD])
                u = sb.tile([128, FD], F32, tag="u")
                R = sb.tile([128, FD], F32, tag="R")
                negf = sb.tile([128, FD], F32, tag="negf")
                lo = sb.tile([128, FD], F16, tag="lo")
                hiF = sb.tile([128, FD], F16, tag="hi")
                # u = x*(255/16) in [0,16); fp32 magic-number floor:
                # R = (u - 0.5) + 1.5*2^23 -> R - MAGIC = floorish(u)
                # (round-half-even at exact integers is absorbed by the
                # tent overlap column)
                nc.vector.tensor_scalar(u[:], xt[:], 255.0 / 16.0, None,
                                        ALU.mult)
                nc.vector.tensor_scalar(R[:], u[:], -0.5, MAGIC, ALU.add,
                                        ALU.add)
                nc.vector.scalar_tensor_tensor(
                    negf[:], R[:], -MAGIC, u[:], ALU.add, ALU.subtract)
                # casts on ACT (Copy allows float bias/scale immediates)
                nc.scalar.activation(lo[:], negf[:], ACTF.Copy, bias=0.0,
                                     scale=-16.0)
                nc.scalar.activation(hiF[:], R[:], ACTF.Copy, bias=-MAGIC,
                                     scale=1.0)
                U = sb.tile([128, FD // G, NA, G], F16, tag="U")
                V = sb.tile([128, FD // G, NB, G], F16, tag="V")
                hiG = hiF[:].rearrange("p (q g) -> p q g", g=G)
                loG = lo[:].rearrange("p (q g) -> p q g", g=G)
                for a in range(NA):
                    nc.vector.tensor_scalar(
                        U[:, :, a, :], hiG, float(a), None, ALU.is_equal)
                for p in range(NB):
                    # ramp column c = p-1: relu(lo - c); tent recovered at
                    # readout via tent(d) = relu(d+1) - 2 relu(d) + relu(d-1)
                    c = p - 1
                    if p < N_V_ACT:
                        nc.scalar.activation(
                            V[:, :, p, :], loG, ACTF.Relu, bias=float(-c),
                            scale=1.0)
                    else:
                        nc.vector.tensor_scalar(
                            V[:, :, p, :], loG, float(c), 0.0,
                            ALU.subtract, ALU.max)
                for q in range(FD // G):
                    nc.tensor.matmul(
                        acc[:],
                        U[:, q].opt(),
                        V[:, q].opt(),
                        start=(t == 0 and q == 0),
                        stop=(t == ntiles - 1 and q == FD // G - 1),
                    )
            res = sbo.tile([128, NOUT], F32)
            nc.vector.tensor_copy(res[:], acc[:])
            nc.sync.dma_start(out[:], res[:])
    _split_excess_waits(nc)
    return nc


_NC_CACHE = None


def _get_nc():
    global _NC_CACHE
    if _NC_CACHE is None:
        _NC_CACHE = _build_nc()
    return _NC_CACHE


def _shard(img):
    flat = np.ascontiguousarray(np.asarray(img, dtype=np.float32)).reshape(-1)
    assert flat.size == NCORES * 128 * NCOLS
    return flat.reshape(NCORES, 128, NCOLS)


def _combine(per_core_hists):
    P = np.zeros((128, NOUT), np.float64)
    for r in per_core_hists:
        P += np.asarray(r, dtype=np.float64)
    R = P.reshape(NA, G, NB, G)
    CR = np.einsum('agbg->ab', R)          # [16, 17] ramp sums, c=-1..15
    CRz = np.concatenate([CR, np.zeros((NA, 2))], axis=1)
    T = CRz[:, 0:17] - 2.0 * CRz[:, 1:18] + CRz[:, 2:19]  # tent sums b=0..16
    h = np.zeros(NA * 16 + 1, np.float64)
    for a in range(NA):
        h[16 * a:16 * a + 16] += T[a, :16]
        h[16 * a + 16] += T[a, 16]
    return h[:256].astype(np.float32)


def kernel(img):
    from concourse.bass_utils import run_bass_kernel_spmd
    shards = _shard(img)
    in_maps = [{"x": shards[i]} for i in range(NCORES)]
    res = run_bass_kernel_spmd(_get_nc(), in_maps, core_ids=list(range(NCORES)))
    return _combine([res.results[i]["hist"] for i in range(NCORES)])



# revision 2
# speedup vs baseline: 1.0147x; 1.0147x over previous
"""Trainium2 Bass kernel for nn_DiffHist (differentiable 256-bin histogram).

Contract: kernel(img) takes the FULL input img [128, 512, 512] f32 with
values in [0, 1], returns the FULL output h[256] f32 — identical math to
the reference:
    s = 255*img.ravel(); idx = floor(s); d = s - idx
    h[idx] += 1-d; h[idx+1] += d; return h[:256]

Data-parallel over 8 NeuronCores; each core gets 1/8 of the flattened
image as a [128, 32768] f32 block.  Per core the histogram is a
PSUM-accumulated bilinear form on the tensor engine:

  u = x*255/16 in [0,16);  a = floor(u) via fp16 magic round
  (R = RNE(u + 1535.5) = 1536 + a);  A = u - a in [0,1)

  M[a, cc] = sum_i [a_i == a] * relu(A_i - cc/16),   cc = -1..15
  h[16a+b] = 16 * (M[a,b-1] - 2 M[a,b] + M[a,b+1])   (tent = 2nd diff)

G=8 chunks of 128 elements are packed block-diagonally per matmul
(lhsT [128,128] one-hot, rhs [128,136] ramps); PSUM accumulates across
all 4096 matmuls.  The a=15 one-hot column is constant 1.0 (written
once per pool buffer); its row is decoded on the host as
(sum over all) - (sum of rows 0..14).

Engine split per [128,1024] f32 tile (HWDGE DMA):
  ACT:  u16 = Copy(x * 255/16) fp16 cast; 7 high ramps relu(A - c)
  DVE:  R = u16 + 1535.5 (magic); A = (u16 + 1536) - R written straight
        into the V tile's cc=0 plane; 15 one-hots is_equal(R, 1536+a);
        ramp cc=-1 as A + 1/16; 7 mid ramps as (A max c) - c
All DVE ops are single-src fp16 (4x mode) except the 2-src A op (2x).

Numerics: fp16 quantization of u dominates: ~2e-4 rel L2 vs the f32
reference — well inside the 2e-2 gate.
"""
import sys

sys.path.insert(0, '/opt/trn_rl_repo')

import numpy as np

# ----------------------------------------------------------------- tile patch
# The pinned walrus build accepts only one sync-wait command on several
# instruction classes; current concourse Tile attaches several to the
# kernel-tail drain and occasionally to DMA ops.  Split the excess waits
# onto dedicated single-wait instructions.
import bass_rust
import concourse.tile as tile
import concourse.mybir as mybir
from bass_rust import ScopedClock

_MAX_WAITS = 1


def _drain_and_barrier_split(self, tick_clock, wait_clock):
    nc = self.nc
    drain_inst = nc.sync.drain()
    wait_clock.add_sem_waits(
        drain_inst.ins, ScopedClock({None: tick_clock.global_clock})
    )
    si = drain_inst.ins.sync_info
    waits = list(si.on_wait) if si is not None and si.on_wait else []
    if len(waits) > _MAX_WAITS:
        drain_inst.ins.sync_info = bass_rust.SyncInfo(
            on_wait=waits[:_MAX_WAITS], on_update=list(si.on_update)
        )
        for w in waits[_MAX_WAITS:]:
            d2 = nc.sync.drain()
            d2.ins.sync_info = bass_rust.SyncInfo(on_wait=[w], on_update=[])
    nc.all_engine_barrier()
    assert self.sems is not None
    popped = nc._tile_sem_poison_stack.pop()
    assert popped is self._sem_poison
    nc.clear_and_free_semaphores(list(self.sems.allocated().values()))
    nc.all_engine_barrier()


def _split_excess_waits(nc, max_waits=_MAX_WAITS):
    for bb in nc.main_func.blocks:
        insts = list(bb.instructions)
        out = []
        changed = False
        for ins in insts:
            si = ins.sync_info
            if si is not None and si.on_wait and len(si.on_wait) > max_waits:
                waits = list(si.on_wait)
                extra, keep = waits[:-max_waits], waits[-max_waits:]
                for w in extra:
                    nop = mybir.InstNoOp(
                        name=f"waitnop-{nc.next_id()}",
                        engine=ins.engine,
                        bass_nofuse=True,
                        sync_info=mybir.SyncInfo(on_wait=[w], on_update=[]),
                    )
                    nc.register_instruction(nop, overwrite=True)
                    out.append(nop)
                ins.sync_info = bass_rust.SyncInfo(
                    on_wait=keep, on_update=list(si.on_update)
                )
                changed = True
            out.append(ins)
        if changed:
            bb.instructions = out


tile.TileContext._drain_and_barrier = _drain_and_barrier_split

# ----------------------------------------------------------------- kernel
import concourse.bass as bass

F32 = mybir.dt.float32
F16 = mybir.dt.float16
ALU = mybir.AluOpType
ACTF = mybir.ActivationFunctionType

NCORES = 8
NCOLS = 32768          # elements per partition per core
NA = 16                # coarse blocks
NB = 17                # ramp columns cc = -1..15 (tent = 2nd diff)
G = 8                  # chunks per matmul
NOUT = NB * G          # 136
FD = 1024              # columns per tile
MAGIC = 1536.0         # 1.5 * 2^10 (fp16 magic round)
N_V_ACT = 7            # high ramp columns on the scalar engine (ACT Relu)


def _build_nc():
    nc = bass.Bass()
    x = nc.declare_dram_parameter("x", [128, NCOLS], F32, isOutput=False)
    out = nc.declare_dram_parameter("hist", [128, NOUT], F32, isOutput=True)
    ntiles = NCOLS // FD

    # const APs for ACT biases (mirrors Bass.__init__ register_const_ap)
    for v in [float(-(p - 1) / 16.0) for p in range(NB - N_V_ACT, NB)]:
        if (F32, v) not in nc.const_aps.aps:
            tcon = nc.alloc_sbuf_tensor(f"const-float32-{v}", [128, 1], F32)
            nc.gpsimd.memset(tcon.ap(), v)
            nc.const_aps.aps[(F32, v)] = tcon.ap()
    nc.all_engine_barrier()

    with tile.TileContext(nc) as tc:
        with (
            tc.tile_pool(name="sb", bufs=2) as sb,
            tc.tile_pool(name="sbo", bufs=1) as sbo,
            tc.tile_pool(name="psum", bufs=1, space="PSUM") as psum,
        ):
            acc = psum.tile([128, NOUT], F32)
            for t in range(ntiles):
                xt = sb.tile([128, FD], F32, tag="x")
                nc.sync.dma_start(xt[:], x[:, t * FD:(t + 1) * FD])
                u16 = sb.tile([128, FD], F16, tag="u16")
                R = sb.tile([128, FD], F16, tag="R")
                U = sb.tile([128, FD // G, NA, G], F16, tag="U")
                V = sb.tile([128, FD // G, NB, G], F16, tag="V")
                # u16 = x*255/16 in [0,16), fp16 (ACT: fused cast+scale)
                nc.scalar.activation(u16[:], xt[:], ACTF.Copy,
                                     bias=0.0, scale=255.0 / 16.0)
                # R = RNE(u + 1535.5) = 1536 + a  (fp16 magic round)
                nc.vector.tensor_scalar(R[:], u16[:], MAGIC - 0.5, None,
                                        ALU.add)
                # A = frac(u) = (u16 + 1536) - R, written into the cc=0
                # ramp plane of V (relu(A - 0) = A since A >= 0)
                AG = V[:, :, 1, :]
                nc.vector.scalar_tensor_tensor(
                    AG, u16[:].rearrange("p (q g) -> p q g", g=G), MAGIC,
                    R[:].rearrange("p (q g) -> p q g", g=G),
                    ALU.add, ALU.subtract)
                RG = R[:].rearrange("p (q g) -> p q g", g=G)
                for a in range(NA - 1):
                    nc.vector.tensor_scalar(
                        U[:, :, a, :], RG, MAGIC + a, None, ALU.is_equal)
                if t < 2:
                    # constant ones column (a=15); pool has 2 buffers, so
                    # write it once in each and never touch it again
                    nc.gpsimd.memset(U[:, :, NA - 1, :], 1.0)
                # ramp cc=-1: relu(A + 1/16) = A + 1/16  (A >= 0)
                nc.vector.tensor_scalar(
                    V[:, :, 0, :], AG, 1.0 / 16.0, None, ALU.add)
                for p in range(2, NB):
                    cc = p - 1
                    sc = float(cc / 16.0)
                    if p < NB - N_V_ACT:
                        # (A max sc) - sc == relu(A - sc), exact in fp16
                        nc.vector.tensor_scalar(
                            V[:, :, p, :], AG, sc, sc, ALU.max, ALU.subtract)
                    else:
                        nc.scalar.activation(
                            V[:, :, p, :], AG, ACTF.Relu, bias=-sc, scale=1.0)
                for q in range(FD // G):
                    nc.tensor.matmul(
                        acc[:],
                        U[:, q].opt(),
                        V[:, q].opt(),
                        start=(t == 0 and q == 0),
                        stop=(t == ntiles - 1 and q == FD // G - 1),
                    )
            res = sbo.tile([128, NOUT], F32)
            nc.vector.tensor_copy(res[:], acc[:])
            nc.sync.dma_start(out[:], res[:])
    _split_excess_waits(nc)
    return nc


_NC_CACHE = None


def _get_nc():
    global _NC_CACHE
    if _NC_CACHE is None:
        _NC_CACHE = _build_nc()
    return _NC_CACHE


def _shard(img):
    flat = np.ascontiguousarray(np.asarray(img, dtype=np.float32)).reshape(-1)
    assert flat.size == NCORES * 128 * NCOLS
    return flat.reshape(NCORES, 128, NCOLS)


def _combine(per_core_hists):
    P = np.zeros((128, NOUT), np.float64)
    for r in per_core_hists:
        P += np.asarray(r, dtype=np.float64)
    Rm = P.reshape(NA, G, NB, G)
    CR = np.einsum('agbg->ab', Rm)         # [16, 17] ramp sums, cc=-1..15
    # a=15 row used a constant-ones lhsT column: it holds the sum over ALL
    # elements; subtract rows 0..14 to recover the true a=15 sums.
    CR[NA - 1] -= CR[:NA - 1].sum(axis=0)
    CR *= 16.0                             # ramps were built on A = lo/16
    CRz = np.concatenate([CR, np.zeros((NA, 2))], axis=1)
    T = CRz[:, 0:17] - 2.0 * CRz[:, 1:18] + CRz[:, 2:19]  # tent sums b=0..16
    h = np.zeros(NA * 16 + 1, np.float64)
    for a in range(NA):
        h[16 * a:16 * a + 16] += T[a, :16]
        h[16 * a + 16] += T[a, 16]
    return h[:256].astype(np.float32)


def kernel(img):
    from concourse.bass_utils import run_bass_kernel_spmd
    shards = _shard(img)
    in_maps = [{"x": shards[i]} for i in range(NCORES)]
    res = run_bass_kernel_spmd(_get_nc(), in_maps, core_ids=list(range(NCORES)))
    return _combine([res.results[i]["hist"] for i in range(NCORES)])


# revision 3
# speedup vs baseline: 1.0265x; 1.0116x over previous
"""Trainium2 Bass kernel for nn_DiffHist (differentiable 256-bin histogram).

Contract: kernel(img) takes the FULL input img [128, 512, 512] f32 with
values in [0, 1], returns the FULL output h[256] f32 — identical math to
the reference:
    s = 255*img.ravel(); idx = floor(s); d = s - idx
    h[idx] += 1-d; h[idx+1] += d; return h[:256]

Data-parallel over 8 NeuronCores; each core gets 1/8 of the flattened
image as a [128, 32768] f32 block.  Per core the histogram is a
PSUM-accumulated bilinear form on the tensor engine:

  u = x*255/16 in [0,16);  a = floor(u) via fp16 magic round
  (R = RNE(u + 1535.5) = 1536 + a);  A = u - a in [0,1)

  lhsT one-hot U[k, a] = [a_k == a] (a=15 column is constant 1.0,
  decoded on the host); rhs V columns: plane0 = constant 1.0 (counts),
  plane1 = A, planes 2..16 = relu(A - cc/16) for cc = 1..15.
  Host readout rebuilds the cc=-1..15 ramp sums
  (M[-1] = 16*M_A + T from the count column) and takes the tent
  second difference.

G=8 chunks of 128 elements are packed block-diagonally per matmul
(lhsT [128,128], rhs [128,136]); PSUM accumulates across all 4096
matmuls.

Scheduling: DMA + ACT cast (u16) + DVE prep (R, A) for tile t+1 are
issued BEFORE tile t's one-hot/ramp work, so neither ACT's ramps nor
DVE's one-hots ever wait on the other engine at a tile boundary.
Engine split per tile: ACT = u16 cast + 7 high ramps; DVE = R, A,
15 one-hots, 8 low ramps ((A max c) - c, exact).
"""
import sys

sys.path.insert(0, '/opt/trn_rl_repo')

import numpy as np

# ----------------------------------------------------------------- tile patch
# The pinned walrus build accepts only one sync-wait command on several
# instruction classes; current concourse Tile attaches several to the
# kernel-tail drain and occasionally to DMA ops.  Split the excess waits
# onto dedicated single-wait instructions.
import bass_rust
import concourse.tile as tile
import concourse.mybir as mybir
from bass_rust import ScopedClock

_MAX_WAITS = 1


def _drain_and_barrier_split(self, tick_clock, wait_clock):
    nc = self.nc
    drain_inst = nc.sync.drain()
    wait_clock.add_sem_waits(
        drain_inst.ins, ScopedClock({None: tick_clock.global_clock})
    )
    si = drain_inst.ins.sync_info
    waits = list(si.on_wait) if si is not None and si.on_wait else []
    if len(waits) > _MAX_WAITS:
        drain_inst.ins.sync_info = bass_rust.SyncInfo(
            on_wait=waits[:_MAX_WAITS], on_update=list(si.on_update)
        )
        for w in waits[_MAX_WAITS:]:
            d2 = nc.sync.drain()
            d2.ins.sync_info = bass_rust.SyncInfo(on_wait=[w], on_update=[])
    nc.all_engine_barrier()
    assert self.sems is not None
    popped = nc._tile_sem_poison_stack.pop()
    assert popped is self._sem_poison
    nc.clear_and_free_semaphores(list(self.sems.allocated().values()))
    nc.all_engine_barrier()


def _split_excess_waits(nc, max_waits=_MAX_WAITS):
    for bb in nc.main_func.blocks:
        insts = list(bb.instructions)
        out = []
        changed = False
        for ins in insts:
            si = ins.sync_info
            if si is not None and si.on_wait and len(si.on_wait) > max_waits:
                waits = list(si.on_wait)
                extra, keep = waits[:-max_waits], waits[-max_waits:]
                for w in extra:
                    nop = mybir.InstNoOp(
                        name=f"waitnop-{nc.next_id()}",
                        engine=ins.engine,
                        bass_nofuse=True,
                        sync_info=mybir.SyncInfo(on_wait=[w], on_update=[]),
                    )
                    nc.register_instruction(nop, overwrite=True)
                    out.append(nop)
                ins.sync_info = bass_rust.SyncInfo(
                    on_wait=keep, on_update=list(si.on_update)
                )
                changed = True
            out.append(ins)
        if changed:
            bb.instructions = out


tile.TileContext._drain_and_barrier = _drain_and_barrier_split

# ----------------------------------------------------------------- kernel
import concourse.bass as bass

F32 = mybir.dt.float32
F16 = mybir.dt.float16
ALU = mybir.AluOpType
ACTF = mybir.ActivationFunctionType

NCORES = 8
NCOLS = 32768          # elements per partition per core
NA = 16                # coarse blocks
NB = 17                # rhs columns: ones, A, ramps cc=1..15
G = 8                  # chunks per matmul
NOUT = NB * G          # 136
FD = 1024              # columns per tile
MAGIC = 1536.0         # 1.5 * 2^10 (fp16 magic round)
N_V_ACT = 7            # high ramp columns on the scalar engine (ACT Relu)


def _build_nc():
    nc = bass.Bass()
    x = nc.declare_dram_parameter("x", [128, NCOLS], F32, isOutput=False)
    out = nc.declare_dram_parameter("hist", [128, NOUT], F32, isOutput=True)
    ntiles = NCOLS // FD

    # const APs for ACT biases (mirrors Bass.__init__ register_const_ap)
    for v in [float(-(p - 1) / 16.0) for p in range(NB - N_V_ACT, NB)]:
        if (F32, v) not in nc.const_aps.aps:
            tcon = nc.alloc_sbuf_tensor(f"const-float32-{v}", [128, 1], F32)
            nc.gpsimd.memset(tcon.ap(), v)
            nc.const_aps.aps[(F32, v)] = tcon.ap()
    nc.all_engine_barrier()

    with tile.TileContext(nc) as tc:
        with (
            tc.tile_pool(name="sb", bufs=2) as sb,
            tc.tile_pool(name="sbo", bufs=1) as sbo,
            tc.tile_pool(name="psum", bufs=1, space="PSUM") as psum,
        ):
            acc = psum.tile([128, NOUT], F32)

            def load_cast(t):
                xt = sb.tile([128, FD], F32, tag="x")
                nc.sync.dma_start(xt[:], x[:, t * FD:(t + 1) * FD])
                u16 = sb.tile([128, FD], F16, tag="u16")
                nc.scalar.activation(u16[:], xt[:], ACTF.Copy,
                                     bias=0.0, scale=255.0 / 16.0)
                return u16

            def prep(t, u16):
                # R = RNE(u + 1535.5) = 1536 + a  (fp16 magic round)
                R = sb.tile([128, FD], F16, tag="R")
                nc.vector.tensor_scalar(R[:], u16[:], MAGIC - 0.5, None,
                                        ALU.add)
                # A = frac(u) = (u16 + 1536) - R, into V's cc=0 plane
                V = sb.tile([128, FD // G, NB, G], F16, tag="V")
                nc.vector.scalar_tensor_tensor(
                    V[:, :, 1, :],
                    u16[:].rearrange("p (q g) -> p q g", g=G), MAGIC,
                    R[:].rearrange("p (q g) -> p q g", g=G),
                    ALU.add, ALU.subtract)
                if t < 2:
                    # constant count column (written once per pool buffer)
                    nc.gpsimd.memset(V[:, :, 0, :], 1.0)
                return R, V

            u16_n = load_cast(0)
            R_c, V_c = prep(0, u16_n)
            for t in range(ntiles):
                R, V = R_c, V_c
                if t + 1 < ntiles:
                    u16_n = load_cast(t + 1)
                    R_c, V_c = prep(t + 1, u16_n)
                U = sb.tile([128, FD // G, NA, G], F16, tag="U")
                RG = R[:].rearrange("p (q g) -> p q g", g=G)
                AG = V[:, :, 1, :]
                for a in range(NA - 1):
                    nc.vector.tensor_scalar(
                        U[:, :, a, :], RG, MAGIC + a, None, ALU.is_equal)
                if t < 2:
                    # constant ones column (a=15), once per pool buffer
                    nc.gpsimd.memset(U[:, :, NA - 1, :], 1.0)
                for p in range(2, NB):
                    cc = p - 1
                    sc = float(cc / 16.0)
                    if p < NB - N_V_ACT:
                        # (A max sc) - sc == relu(A - sc), exact in fp16
                        nc.vector.tensor_scalar(
                            V[:, :, p, :], AG, sc, sc, ALU.max, ALU.subtract)
                    else:
                        nc.scalar.activation(
                            V[:, :, p, :], AG, ACTF.Relu, bias=-sc, scale=1.0)
                for q in range(FD // G):
                    nc.tensor.matmul(
                        acc[:],
                        U[:, q].opt(),
                        V[:, q].opt(),
                        start=(t == 0 and q == 0),
                        stop=(t == ntiles - 1 and q == FD // G - 1),
                    )
            res = sbo.tile([128, NOUT], F32)
            nc.vector.tensor_copy(res[:], acc[:])
            nc.sync.dma_start(out[:], res[:])
    _split_excess_waits(nc)
    return nc


_NC_CACHE = None


def _get_nc():
    global _NC_CACHE
    if _NC_CACHE is None:
        _NC_CACHE = _build_nc()
    return _NC_CACHE


def _shard(img):
    flat = np.ascontiguousarray(np.asarray(img, dtype=np.float32)).reshape(-1)
    assert flat.size == NCORES * 128 * NCOLS
    return flat.reshape(NCORES, 128, NCOLS)


def _combine(per_core_hists):
    P = np.zeros((128, NOUT), np.float64)
    for r in per_core_hists:
        P += np.asarray(r, dtype=np.float64)
    Rm = P.reshape(NA, G, NB, G)
    CR = np.einsum('agbg->ab', Rm)     # [16, 17]: counts, sum(A), ramp sums
    # a=15 row used a constant-ones lhsT column: it holds the sum over ALL
    # elements; subtract rows 0..14 to recover the true a=15 sums.
    CR[NA - 1] -= CR[:NA - 1].sum(axis=0)
    # rebuild ramp sums M[a, cc] = sum [a_i==a] relu(lo_i - cc), cc=-1..15
    # (lo = 16*A): cc=-1 column = 16*sum(A) + count; cc>=0 from cols 1..16
    Mt = np.zeros((NA, 17))
    Mt[:, 0] = 16.0 * CR[:, 1] + CR[:, 0]
    Mt[:, 1:17] = 16.0 * CR[:, 1:17]
    CRz = np.concatenate([Mt, np.zeros((NA, 2))], axis=1)
    T = CRz[:, 0:17] - 2.0 * CRz[:, 1:18] + CRz[:, 2:19]  # tent sums b=0..16
    h = np.zeros(NA * 16 + 1, np.float64)
    for a in range(NA):
        h[16 * a:16 * a + 16] += T[a, :16]
        h[16 * a + 16] += T[a, 16]
    return h[:256].astype(np.float32)


def kernel(img):
    from concourse.bass_utils import run_bass_kernel_spmd
    shards = _shard(img)
    in_maps = [{"x": shards[i]} for i in range(NCORES)]
    res = run_bass_kernel_spmd(_get_nc(), in_maps, core_ids=list(range(NCORES)))
    return _combine([res.results[i]["hist"] for i in range(NCORES)])


# revision 4
# speedup vs baseline: 1.0370x; 1.0102x over previous
"""Trainium2 Bass kernel for nn_DiffHist (differentiable 256-bin histogram).

Contract: kernel(img) takes the FULL input img [128, 512, 512] f32 with
values in [0, 1], returns the FULL output h[256] f32 — identical math to
the reference:
    s = 255*img.ravel(); idx = floor(s); d = s - idx
    h[idx] += 1-d; h[idx+1] += d; return h[:256]

Data-parallel over 8 NeuronCores; each core gets 1/8 of the flattened
image as a [128, 32768] f32 block.  Per core the histogram is a
PSUM-accumulated bilinear form on the tensor engine:

  u = x*255/16 in [0,16);  a = floor(u) via fp16 magic round
  (R = RNE(u + 1535.5) = 1536 + a);  A = u - a in [0,1)

  lhsT one-hot U[k, a] = [a_k == a] (a=15 column is constant 1.0,
  decoded on the host); rhs V columns: plane0 = constant 1.0 (counts),
  plane1 = A, planes 2..16 = relu(A - cc/16) for cc = 1..15.
  Host readout rebuilds the cc=-1..15 ramp sums
  (M[-1] = 16*M_A + T from the count column) and takes the tent
  second difference.

G=8 chunks of 128 elements are packed block-diagonally per matmul
(lhsT [128,128], rhs [128,136]); PSUM accumulates across all 4096
matmuls.

Scheduling: DMA + ACT cast (u16) + DVE prep (R, A) for tile t+1 are
issued BEFORE tile t's one-hot/ramp work, so neither ACT's ramps nor
DVE's one-hots ever wait on the other engine at a tile boundary.
Engine split per tile: ACT = u16 cast + 7 high ramps; DVE = R, A,
15 one-hots, 8 low ramps ((A max c) - c, exact).
"""
import sys

sys.path.insert(0, '/opt/trn_rl_repo')

import numpy as np

# ----------------------------------------------------------------- tile patch
# The pinned walrus build accepts only one sync-wait command on several
# instruction classes; current concourse Tile attaches several to the
# kernel-tail drain and occasionally to DMA ops.  Split the excess waits
# onto dedicated single-wait instructions.
import bass_rust
import concourse.tile as tile
import concourse.mybir as mybir
from bass_rust import ScopedClock

_MAX_WAITS = 1


def _drain_and_barrier_split(self, tick_clock, wait_clock):
    nc = self.nc
    drain_inst = nc.sync.drain()
    wait_clock.add_sem_waits(
        drain_inst.ins, ScopedClock({None: tick_clock.global_clock})
    )
    si = drain_inst.ins.sync_info
    waits = list(si.on_wait) if si is not None and si.on_wait else []
    if len(waits) > _MAX_WAITS:
        drain_inst.ins.sync_info = bass_rust.SyncInfo(
            on_wait=waits[:_MAX_WAITS], on_update=list(si.on_update)
        )
        for w in waits[_MAX_WAITS:]:
            d2 = nc.sync.drain()
            d2.ins.sync_info = bass_rust.SyncInfo(on_wait=[w], on_update=[])
    nc.all_engine_barrier()
    assert self.sems is not None
    popped = nc._tile_sem_poison_stack.pop()
    assert popped is self._sem_poison
    nc.clear_and_free_semaphores(list(self.sems.allocated().values()))
    nc.all_engine_barrier()


def _split_excess_waits(nc, max_waits=_MAX_WAITS):
    for bb in nc.main_func.blocks:
        insts = list(bb.instructions)
        out = []
        changed = False
        for ins in insts:
            si = ins.sync_info
            if si is not None and si.on_wait and len(si.on_wait) > max_waits:
                waits = list(si.on_wait)
                extra, keep = waits[:-max_waits], waits[-max_waits:]
                for w in extra:
                    nop = mybir.InstNoOp(
                        name=f"waitnop-{nc.next_id()}",
                        engine=ins.engine,
                        bass_nofuse=True,
                        sync_info=mybir.SyncInfo(on_wait=[w], on_update=[]),
                    )
                    nc.register_instruction(nop, overwrite=True)
                    out.append(nop)
                ins.sync_info = bass_rust.SyncInfo(
                    on_wait=keep, on_update=list(si.on_update)
                )
                changed = True
            out.append(ins)
        if changed:
            bb.instructions = out


tile.TileContext._drain_and_barrier = _drain_and_barrier_split

# ----------------------------------------------------------------- kernel
import concourse.bass as bass

F32 = mybir.dt.float32
F16 = mybir.dt.float16
ALU = mybir.AluOpType
ACTF = mybir.ActivationFunctionType

NCORES = 8
NCOLS = 32768          # elements per partition per core
NA = 16                # coarse blocks
NB = 17                # rhs columns: ones, A, ramps cc=1..15
G = 8                  # chunks per matmul
NOUT = NB * G          # 136
FD = 1024              # columns per tile
MAGIC = 1536.0         # 1.5 * 2^10 (fp16 magic round)
N_V_ACT = 7            # high ramp columns on the scalar engine (ACT Relu)


def _build_nc():
    nc = bass.Bass()
    x = nc.declare_dram_parameter("x", [128, NCOLS], F32, isOutput=False)
    out = nc.declare_dram_parameter("hist", [128, NOUT], F32, isOutput=True)
    ntiles = NCOLS // FD

    # const APs for ACT biases (mirrors Bass.__init__ register_const_ap)
    for v in [float(-(p - 1) / 16.0) for p in range(NB - N_V_ACT, NB)]:
        if (F32, v) not in nc.const_aps.aps:
            tcon = nc.alloc_sbuf_tensor(f"const-float32-{v}", [128, 1], F32)
            nc.gpsimd.memset(tcon.ap(), v)
            nc.const_aps.aps[(F32, v)] = tcon.ap()
    nc.all_engine_barrier()

    with tile.TileContext(nc) as tc:
        with (
            tc.tile_pool(name="sb", bufs=2) as sb,
            tc.tile_pool(name="sbo", bufs=1) as sbo,
            tc.tile_pool(name="psum", bufs=1, space="PSUM") as psum,
        ):
            acc = psum.tile([128, NOUT], F32)

            def load_cast(t):
                xt = sb.tile([128, FD], F32, tag="x")
                nc.sync.dma_start(xt[:], x[:, t * FD:(t + 1) * FD])
                u16 = sb.tile([128, FD], F16, tag="u16")
                nc.scalar.activation(u16[:], xt[:], ACTF.Copy,
                                     bias=0.0, scale=255.0 / 16.0)
                return u16

            def prep(t, u16):
                # R = RNE(u + 1535.5) = 1536 + a  (fp16 magic round)
                R = sb.tile([128, FD], F16, tag="R")
                nc.vector.tensor_scalar(R[:], u16[:], MAGIC - 0.5, None,
                                        ALU.add)
                # R2 = a (exact small integer), then A = u16 - a into V's
                # cc=0 plane; ts at 4x + tt at 2x beats the 1x-mode stt
                R2 = sb.tile([128, FD], F16, tag="R2")
                nc.vector.tensor_scalar(R2[:], R[:], -MAGIC, None, ALU.add)
                V = sb.tile([128, FD // G, NB, G], F16, tag="V")
                nc.vector.tensor_tensor(
                    V[:, :, 1, :],
                    u16[:].rearrange("p (q g) -> p q g", g=G),
                    R2[:].rearrange("p (q g) -> p q g", g=G),
                    ALU.subtract)
                if t < 2:
                    # constant count column (written once per pool buffer)
                    nc.gpsimd.memset(V[:, :, 0, :], 1.0)
                return R, V

            u16_n = load_cast(0)
            R_c, V_c = prep(0, u16_n)
            for t in range(ntiles):
                R, V = R_c, V_c
                if t + 1 < ntiles:
                    u16_n = load_cast(t + 1)
                    R_c, V_c = prep(t + 1, u16_n)
                U = sb.tile([128, FD // G, NA, G], F16, tag="U")
                RG = R[:].rearrange("p (q g) -> p q g", g=G)
                AG = V[:, :, 1, :]
                for a in range(NA - 1):
                    nc.vector.tensor_scalar(
                        U[:, :, a, :], RG, MAGIC + a, None, ALU.is_equal)
                if t < 2:
                    # constant ones column (a=15), once per pool buffer
                    nc.gpsimd.memset(U[:, :, NA - 1, :], 1.0)
                for p in range(2, NB):
                    cc = p - 1
                    sc = float(cc / 16.0)
                    if p < NB - N_V_ACT:
                        # (A max sc) - sc == relu(A - sc), exact in fp16
                        nc.vector.tensor_scalar(
                            V[:, :, p, :], AG, sc, sc, ALU.max, ALU.subtract)
                    else:
                        nc.scalar.activation(
                            V[:, :, p, :], AG, ACTF.Relu, bias=-sc, scale=1.0)
                for q in range(FD // G):
                    nc.tensor.matmul(
                        acc[:],
                        U[:, q].opt(),
                        V[:, q].opt(),
                        start=(t == 0 and q == 0),
                        stop=(t == ntiles - 1 and q == FD // G - 1),
                    )
            res = sbo.tile([128, NOUT], F32)
            nc.vector.tensor_copy(res[:], acc[:])
            nc.sync.dma_start(out[:], res[:])
    _split_excess_waits(nc)
    return nc


_NC_CACHE = None


def _get_nc():
    global _NC_CACHE
    if _NC_CACHE is None:
        _NC_CACHE = _build_nc()
    return _NC_CACHE


def _shard(img):
    flat = np.ascontiguousarray(np.asarray(img, dtype=np.float32)).reshape(-1)
    assert flat.size == NCORES * 128 * NCOLS
    return flat.reshape(NCORES, 128, NCOLS)


def _combine(per_core_hists):
    P = np.zeros((128, NOUT), np.float64)
    for r in per_core_hists:
        P += np.asarray(r, dtype=np.float64)
    Rm = P.reshape(NA, G, NB, G)
    CR = np.einsum('agbg->ab', Rm)     # [16, 17]: counts, sum(A), ramp sums
    # a=15 row used a constant-ones lhsT column: it holds the sum over ALL
    # elements; subtract rows 0..14 to recover the true a=15 sums.
    CR[NA - 1] -= CR[:NA - 1].sum(axis=0)
    # rebuild ramp sums M[a, cc] = sum [a_i==a] relu(lo_i - cc), cc=-1..15
    # (lo = 16*A): cc=-1 column = 16*sum(A) + count; cc>=0 from cols 1..16
    Mt = np.zeros((NA, 17))
    Mt[:, 0] = 16.0 * CR[:, 1] + CR[:, 0]
    Mt[:, 1:17] = 16.0 * CR[:, 1:17]
    CRz = np.concatenate([Mt, np.zeros((NA, 2))], axis=1)
    T = CRz[:, 0:17] - 2.0 * CRz[:, 1:18] + CRz[:, 2:19]  # tent sums b=0..16
    h = np.zeros(NA * 16 + 1, np.float64)
    for a in range(NA):
        h[16 * a:16 * a + 16] += T[a, :16]
        h[16 * a + 16] += T[a, 16]
    return h[:256].astype(np.float32)


def kernel(img):
    from concourse.bass_utils import run_bass_kernel_spmd
    shards = _shard(img)
    in_maps = [{"x": shards[i]} for i in range(NCORES)]
    res = run_bass_kernel_spmd(_get_nc(), in_maps, core_ids=list(range(NCORES)))
    return _combine([res.results[i]["hist"] for i in range(NCORES)])
